# revision 30
# baseline (speedup 1.0000x reference)
"""Distributed Trainium2 kernel for the pairwise-distance alignment loss.

Math (per loss pair (x, y), scale s = 1/(tau*sqrt(D))):
    pos_i  = s*||x_i - y_i||
    dm_ij  = s*||x_i - y_j||
    loss   = mean_i( pos_i - log(sum_j exp(dm_ij)) )
computed for y = label_prompt_embedding (center) and y = aug_x (instance).

Distribution: shard the N=1024 rows of x across 8 NeuronCores (128 rows
each); every core holds the full y (replicated) and computes its
[128, 1024] block of each pairwise matrix, reducing rows locally.

Device algorithm (per core), using the Gram trick
    s^2*||x_i - y_j||^2 = s^2*xsq_i + s^2*ysq_j - 2*s^2*x_i.y_j:
  - rank-1 matmul (bf16): PSUM  = ones^T @ (s^2*ysq)      (K=1)
  - main matmul (bf16):   PSUM += ((-2*s^2*x)^T)^T @ y^T  (K=128)
  - ACT pass 1: t = Ln(PSUM + bias), bias_i = s^2*xsq_i + eps (f32,
    per-partition)
  - ACT pass 2: dm = Exp(0.5*t)       == sqrt of the biased square
  - ACT pass 3: E = Exp(dm), accum_out -> row sums (denom) per y
  sqrt is computed as exp(0.5*ln(v)) so the whole kernel needs a single
  ACT table set (natural_log_exp_and_others), loaded at ACT stream start
  (hoisted by a dummy first activation) under the input DMAs.

Raw Bass (no Tile): tiny engine streams with manual semaphores. The
rank-1 matmuls run first in each PSUM accumulation group because the
small ysq row lands on-chip long before the big y panels. Host prepares
transposed/pre-scaled bf16 operands (O(N*D) work) and does the O(N)
epilogue: log of the denominators, pos terms, final means.
"""

import numpy as np
import ml_dtypes

import concourse.bass as bass
import concourse.mybir as mybir
from concourse import bacc
from concourse.bass_utils import run_bass_kernel_spmd

BF16 = ml_dtypes.bfloat16

N, D, NCORES = 1024, 128, 8
ROWS = N // NCORES          # 128 rows of x per core
TAU, BETA = 1.0, 1.0
S2 = 1.0 / (TAU * TAU * D)  # scale^2
EPS = 1e-3                  # guards ln() against tiny negative Gram residuals

# Strip the unconditional Bass preamble (const-pool memsets + an
# all-engine barrier) from the compiled BIR: this kernel references no
# const APs, and the NRT model-start barrier already fences the engines.
STRIP_PREAMBLE = True

_NC_CACHE = None

# The greedy ACT table-set chooser picks the first set containing each
# activation's function, which thrashes between `natural_log` (Ln) and
# `exp_and_others` (Exp) -- ~1.3us per reload. Both functions live in
# `natural_log_exp_and_others`; steer the chooser there by hiding that
# set's functions from every other set (indices/ids stay untouched).
_COMBINED_SET = "natural_log_exp_and_others"


def _patched_get_activation_tables(arch):
    from concourse.hw_specs import get_activation_tables as _orig

    tabs = _orig(arch)
    target = tabs[_COMBINED_SET]
    return {
        name: (funcs if name == _COMBINED_SET else funcs - target)
        for name, funcs in tabs.items()
    }


def _build(with_dummy=True):
    f32 = mybir.dt.float32
    bf16 = mybir.dt.bfloat16
    AF = mybir.ActivationFunctionType
    nc = bacc.Bacc("TRN2", target_bir_lowering=False, debug=False,
                   num_devices=NCORES)

    # xy = [ (-2*s^2*x)^T | label^T | aug^T ]  (bf16, partition dim = D)
    xy_d = nc.dram_tensor("xy", [D, ROWS + 2 * N], bf16,
                          kind="ExternalInput")
    # q = [ s^2*ysq (2N) | ones (ROWS) ]  (bf16, single partition row)
    q_d = nc.dram_tensor("q", [1, 2 * N + ROWS], bf16, kind="ExternalInput")
    # b = [ s^2*xsq + eps | zeros ]  (f32, per-partition activation biases)
    b_d = nc.dram_tensor("b", [ROWS, 2], f32, kind="ExternalInput")
    out0_d = nc.dram_tensor("out0", [ROWS, 1], f32, kind="ExternalOutput")
    out1_d = nc.dram_tensor("out1", [ROWS, 1], f32, kind="ExternalOutput")

    # staged input pieces: [x block + center-y half | center-y half |
    # instance-y] so the first main matmul starts as early as possible
    P1 = ROWS + 512
    P2 = ROWS + N

    with (
        nc.sbuf_tensor("xy_sb", [D, ROWS + 2 * N], bf16) as xy,
        nc.sbuf_tensor("q_sb", [1, 2 * N + ROWS], bf16) as q,
        nc.sbuf_tensor("b_sb", [ROWS, 2], f32) as b,
        nc.sbuf_tensor("t1_sb", [ROWS, N], f32) as t1,
        nc.sbuf_tensor("t2_sb", [ROWS, N], f32) as t2,
        nc.sbuf_tensor("den_sb", [ROWS, 2], f32) as den,
        nc.psum_tensor("psA", [ROWS, N], f32) as psA,
        nc.psum_tensor("psB", [ROWS, N], f32) as psB,
        nc.semaphore("s_i1") as s_i1,
        nc.semaphore("s_i2") as s_i2,
        nc.semaphore("s_i3") as s_i3,
        nc.semaphore("s_q") as s_q,
        nc.semaphore("s_bias") as s_bias,
        nc.semaphore("s_mm") as s_mm,
        nc.semaphore("s_c") as s_c,
        nc.semaphore("s_out") as s_out,
        nc.Block() as block,
    ):
        xt = xy[:, 0:ROWS]          # lhsT for the main matmuls
        ones = q[:, 2 * N:2 * N + ROWS]  # lhsT for the rank-1 matmuls

        @block.sync
        def _(sync):
            # NOTE: same-queue DMAs can complete out of order -- each
            # piece gets its own semaphore.
            sync.dma_start(q[:], q_d[:]).then_inc(s_q, 16)
            sync.dma_start(xy[:, 0:P1], xy_d[:, 0:P1]).then_inc(s_i1, 16)
            sync.dma_start(xy[:, P1:P2], xy_d[:, P1:P2]).then_inc(s_i2, 16)
            sync.dma_start(xy[:, P2:], xy_d[:, P2:]).then_inc(s_i3, 16)
            # outputs leave as soon as each denominator is read; no
            # completion wait -- the Block-exit drain covers the queue.
            sync.wait_ge(s_c, 3)
            sync.dma_start(out0_d[:], den[:, 0:1]).then_inc(s_out, 16)
            sync.wait_ge(s_c, 6)
            sync.dma_start(out1_d[:], den[:, 1:2]).then_inc(s_out, 16)

        @block.gpsimd
        def _(gpsimd):
            gpsimd.dma_start(b[:], b_d[:]).then_inc(s_bias, 16)

        @block.tensor
        def _(tensor):
            # rank-1 ysq updates first (q lands well before the y panels)
            tensor.wait_ge(s_q, 16)
            for c, ps in ((0, psA), (1, psB)):
                for h in range(2):
                    qsl = slice(c * N + h * 512, c * N + (h + 1) * 512)
                    osl = slice(h * 512, (h + 1) * 512)
                    tensor.matmul(ps[:, osl], ones, q[:, qsl],
                                  start=True, stop=False,
                                  skip_group_check=True)
                for h in range(2):
                    ysl = slice(ROWS + c * N + h * 512,
                                ROWS + c * N + (h + 1) * 512)
                    osl = slice(h * 512, (h + 1) * 512)
                    if c == 0:
                        tensor.wait_ge((s_i1, s_i2)[h], 16)
                    elif h == 0:
                        tensor.wait_ge(s_i3, 16)
                    mm = tensor.matmul(ps[:, osl], xt, xy[:, ysl],
                                       start=False, stop=True,
                                       skip_group_check=True)
                mm.then_inc(s_mm)

        @block.scalar
        def _(scalar):
            bias = b[:, 0:1]
            zero = b[:, 1:2]
            # Dummy first activation with no data deps: hoists the
            # auto-inserted ACT_TABLE_LOAD to stream start, hiding the
            # ~1.3us load under the input DMAs. (Copy keeps its float
            # bias -- no const-AP machinery.)
            if with_dummy:
                scalar.copy(t1[0:1, 0:1], t1[0:1, 0:1])
            scalar.wait_ge(s_bias, 16)
            # ACT is deeply pipelined; explicit waits between the
            # dependent same-engine passes measured consistently faster
            # than relying on in-order issue.
            kk = 0
            for c, ps in ((0, psA), (1, psB)):
                scalar.wait_ge(s_mm, c + 1)
                if kk:
                    scalar.wait_ge(s_c, kk)
                scalar.activation(t1[:], ps[:], AF.Ln,
                                  bias=bias).then_inc(s_c)
                kk += 1
                scalar.wait_ge(s_c, kk)
                scalar.activation(t2[:], t1[:], AF.Exp, bias=zero,
                                  scale=0.5).then_inc(s_c)
                kk += 1
                scalar.wait_ge(s_c, kk)
                scalar.activation(t1[:], t2[:], AF.Exp, bias=zero,
                                  accum_out=den[:, c:c + 1]).then_inc(s_c)
                kk += 1

    _orig_tables = bacc.get_activation_tables
    bacc.get_activation_tables = _patched_get_activation_tables
    try:
        nc.compile()
    finally:
        bacc.get_activation_tables = _orig_tables

    if STRIP_PREAMBLE:
        main = nc.main_func.blocks[0]
        drop = {mybir.InstMemset, mybir.InstDrain, mybir.InstEventSemaphore}
        main.instructions[:] = [
            i for i in main.instructions if type(i) not in drop
        ]
    return nc


def _get_nc():
    global _NC_CACHE
    if _NC_CACHE is None:
        _NC_CACHE = _build()
    return _NC_CACHE


def _prep_in_maps(x, aug, lab):
    s2 = np.float32(S2)
    xT2 = np.ascontiguousarray((x * (-2.0 * s2)).T).astype(BF16)  # [D, N]
    yT = np.ascontiguousarray(np.concatenate([lab, aug], axis=0).T
                              ).astype(BF16)                      # [D, 2N]
    ysq_row = (s2 * np.concatenate([(lab * lab).sum(1), (aug * aug).sum(1)])
               ).astype(BF16)
    q = np.concatenate([ysq_row, np.ones(ROWS, BF16)])[None, :]
    xsqb = (s2 * (x * x).sum(1) + np.float32(EPS)).astype(np.float32)
    b = np.stack([xsqb, np.zeros(N, np.float32)], axis=1)         # [N, 2]

    return [
        {
            "xy": np.ascontiguousarray(
                np.concatenate([xT2[:, k * ROWS:(k + 1) * ROWS], yT], axis=1)),
            "q": q,
            "b": np.ascontiguousarray(b[k * ROWS:(k + 1) * ROWS]),
        }
        for k in range(NCORES)
    ]


def kernel(x, aug_x, label_prompt_embedding):
    x = np.asarray(x, dtype=np.float32)
    aug = np.asarray(aug_x, dtype=np.float32)
    lab = np.asarray(label_prompt_embedding, dtype=np.float32)

    in_maps = _prep_in_maps(x, aug, lab)
    nc = _get_nc()
    res = run_bass_kernel_spmd(nc, in_maps, list(range(NCORES))).results
    den = np.concatenate(
        [np.concatenate([res[k]["out0"], res[k]["out1"]], axis=1)
         for k in range(NCORES)], axis=0)
    lnden = np.log(den)

    # Host epilogue: positive-pair distances and final means (O(N*D)).
    s = np.float32(1.0 / (TAU * np.sqrt(np.float32(D))))
    pos_c = np.sqrt(((x - lab) ** 2).sum(1)) * s
    pos_i = np.sqrt(((x - aug) ** 2).sum(1)) * s
    center = np.float32((pos_c - lnden[:, 0]).mean())
    inst = np.float32((pos_i - lnden[:, 1]).mean())
    total = np.float32(center + np.float32(BETA) * inst)
    return (total, center, inst)
